# revision 34
# baseline (speedup 1.0000x reference)
"""Trainium2 Bass kernel for nn_MultiHeadBlock (B=4, S=2048, D=512, H=8).

Sharding: 8 cores = 4 batches x 2 query-halves. Each core computes K/V for its
batch's full 2048-key sequence (duplicated across the pair of cores sharing a
batch; no collectives), and runs all 8 heads for its 1024 queries.

v2 design (ACT-exp is the per-core floor at ~110us: 16.8M exp elems at
1 elem/cycle/lane @ 1.2GHz):
  - all matmul operands bf16 (host ships x^T / W_qkv / W_o pre-converted):
    warm-PE 1 cycle/row, FWL weight loads, half DMA bytes, 2-4x DVE ops
  - score matmuls for the two heads of a pair interleaved slot-by-slot in
    partition halves 0-63/64-127 so they run concurrently (row tiling)
  - exp instructions fused across both heads: [128, 3*512] PSUM score groups
    -> single ACTIVATE -> bf16 ats in SBUF
  - AV keeps the per-head masked-ones column (sumexp rides the accumulator)
  - softmax normalization: batched reciprocal_approx_fast on [8,512] sumexp
    rows + gpsimd partition_broadcast (idle engine) for the per-query scale
  - PSUM: 6 banks rotate [128,1536] tiles shared by score groups AND QKV
    projection chains (3x512 each) AND the qt-end O-proj/transpose tiles;
    2 banks hold the per-pair AV accumulators. QKV chains are interleaved
    between attention groups of the previous pair so ACT never starves and
    PE stays dense (HAM K=8/8)
"""

import os
import sys

for _p in ("/opt/trn_rl_repo", "/root/.axon_site/_ro/trn_rl_repo"):
    if os.path.isdir(_p) and _p not in sys.path:
        sys.path.insert(0, _p)

import numpy as np

import concourse.bass as bass
import concourse.bacc as bacc
import concourse.mybir as mybir
import concourse.tile as tile
from concourse.masks import make_identity

F32 = mybir.dt.float32
F32R = mybir.dt.float32r
BF16 = mybir.dt.bfloat16
ALU = mybir.AluOpType
ACTF = mybir.ActivationFunctionType

B, S, D = 4, 2048, 512
H, DH = 8, 64
SQ = S // 2          # queries per core
NKT = S // 128       # 16 key row-tiles
NDC = D // 128       # 4 contraction chunks
EPS = 1e-5
N_CORES = 8

# (e, kc) slots in e-interleaved order, grouped 3 per exp instruction
_SLOTS = [(s % 2, s // 2) for s in range(2 * NKT)]
_GROUPS = [_SLOTS[i:i + 3] for i in range(0, 2 * NKT, 3)]


def build_program(probes=False):
    nc = bacc.Bacc("TRN2", target_bir_lowering=False, debug=False,
                   num_devices=N_CORES)

    xt_d = nc.dram_tensor("xt", [D, S], BF16, kind="ExternalInput").ap()
    wqkv_d = nc.dram_tensor("wqkv", [D, 3 * D], BF16, kind="ExternalInput").ap()
    bqkv_d = nc.dram_tensor("bqkv_pt", [128, 12], F32, kind="ExternalInput").ap()
    bvrow_d = nc.dram_tensor("bv_row", [1, D], F32, kind="ExternalInput").ap()
    wo_d = nc.dram_tensor("wo", [D, D], BF16, kind="ExternalInput").ap()
    borow_d = nc.dram_tensor("bo_row", [1, D], F32, kind="ExternalInput").ap()
    gam_d = nc.dram_tensor("gamma_row", [1, D], F32, kind="ExternalInput").ap()
    bet_d = nc.dram_tensor("beta_row", [1, D], F32, kind="ExternalInput").ap()
    maskf_d = nc.dram_tensor("maskf_pt", [128, NKT], F32, kind="ExternalInput").ap()
    out_d = nc.dram_tensor("out", [SQ, D], F32, kind="ExternalOutput").ap()
    dbg = {}
    if probes:
        for nm, shape, dt_ in [("dbg_qt", [128, SQ], BF16),
                               ("dbg_kt", [128, S], BF16),
                               ("dbg_va", [128, 520], BF16),
                               ("dbg_at", [128, 1536], BF16),
                               ("dbg_au", [64, 512], BF16),
                               ("dbg_rs", [1, 512], F32),
                               ("dbg_chunk", [128, 512], BF16),
                               ("dbg_anat", [128, 512], BF16),
                               ("dbg_x", [128, 512], BF16)]:
            dbg[nm] = nc.dram_tensor(nm, shape, dt_,
                                     kind="ExternalOutput").ap()

    with tile.TileContext(nc) as tc:
        with tc.tile_pool(name="const", bufs=1) as cp:
            ident_f = cp.tile([128, 128], F32, name="ident_f")
            make_identity(nc, ident_f[:])
            ident = cp.tile([128, 128], BF16, name="ident")
            nc.vector.tensor_copy(ident[:], ident_f[:])
            ones8 = cp.tile([128, 8], F32, name="ones8")
            nc.vector.memset(ones8[:], 1.0)

            bqkv_sb = cp.tile([128, 12], F32, name="bqkv_sb")
            nc.sync.dma_start(out=bqkv_sb[:], in_=bqkv_d)
            maskf_sb = cp.tile([128, NKT], F32, name="maskf_sb")
            nc.sync.dma_start(out=maskf_sb[:], in_=maskf_d)

            rows = cp.tile([1, 4 * D], F32, name="rows")
            nc.sync.dma_start(out=rows[0:1, 0:D], in_=bvrow_d)
            nc.sync.dma_start(out=rows[0:1, D:2 * D], in_=borow_d)
            nc.sync.dma_start(out=rows[0:1, 2 * D:3 * D], in_=gam_d)
            nc.sync.dma_start(out=rows[0:1, 3 * D:4 * D], in_=bet_d)

            bv_bc = cp.tile([128, D], F32, name="bv_bc")
            bo_b = cp.tile([128, D], BF16, name="bo_b")
            gam_b = cp.tile([128, D], BF16, name="gam_b")
            bet_b = cp.tile([128, D], BF16, name="bet_b")
            tmp_bc = cp.tile([128, D], F32, name="tmp_bc")
            nc.gpsimd.partition_broadcast(bv_bc[:], rows[0:1, 0:D],
                                          channels=128)
            for j, t in enumerate((bo_b, gam_b, bet_b)):
                nc.gpsimd.partition_broadcast(
                    tmp_bc[:], rows[0:1, (j + 1) * D:(j + 2) * D],
                    channels=128)
                nc.vector.tensor_copy(t[:], tmp_bc[:])

            # PE warmup: serialized matmuls covering the input-DMA window so
            # HAM is warm when real matmuls start (free=256 so the stream
            # still spans ~11us after HAM flips to 2.4GHz)
            wrm = cp.tile([128, 256], BF16, name="wrm")
            nc.vector.memset(wrm[:], 0.125)
            with tc.tile_pool(name="warm", bufs=1, space="PSUM") as warmp:
                wps = warmp.tile([128, 256], F32, name="wps")
                for _ in range(40):
                    nc.tensor.matmul(wps[:], lhsT=ident[:], rhs=wrm[:],
                                     start=True, stop=True)

            # input DMAs: spread across the three DMA-capable engine queues
            # AND ordered so the first-needed data (Q/K weight cols, first
            # xt column block) lands first -> QKV chains start ~5us in
            wq_sb = [cp.tile([128, 3 * D], BF16, name=f"wq{dc}")
                     for dc in range(NDC)]
            xt_sb = [cp.tile([128, S], BF16, name=f"xt{dc}")
                     for dc in range(NDC)]
            wo_sb = [cp.tile([128, D], BF16, name=f"wo{c}")
                     for c in range(NDC)]
            # >=2KB per partition line for full DMA rate (bf16: >=1024 cols)
            for dc in range(NDC):       # all wqkv cols, 3KB lines
                nc.gpsimd.dma_start(
                    out=wq_sb[dc][:],
                    in_=wqkv_d[dc * 128:(dc + 1) * 128, :])
            for cb in range(2):         # xt in 1024-col blocks, first half
                for dc in range(NDC):   # (queries 0-1023/keys 0-1023) first
                    eng = nc.sync if dc % 2 == 0 else nc.scalar
                    eng.dma_start(
                        out=xt_sb[dc][:, cb * 1024:(cb + 1) * 1024],
                        in_=xt_d[dc * 128:(dc + 1) * 128,
                                 cb * 1024:(cb + 1) * 1024])
            for c in range(NDC):        # W_o needed only at qt-end
                nc.gpsimd.dma_start(out=wo_sb[c][:],
                                    in_=wo_d[c * 128:(c + 1) * 128, :])

            q_t = [cp.tile([128, SQ], BF16, name=f"qt{t4}") for t4 in range(4)]
            k_t = [cp.tile([128, S], BF16, name=f"kt{t4}") for t4 in range(4)]
            v_aug = [cp.tile([128, H * (DH + 1)], BF16, name=f"va{t}")
                     for t in range(NKT)]
            x_sb = [cp.tile([128, D], BF16, name=f"x{i}") for i in range(8)]
            sumx8 = cp.tile([128, 8], F32, name="sumx8")
            sumsq8 = cp.tile([128, 8], F32, name="sumsq8")
            mu8 = cp.tile([128, 8], F32, name="mu8")
            var8 = cp.tile([128, 8], F32, name="var8")
            rstd8 = cp.tile([128, 8], F32, name="rstd8")


            with tc.tile_pool(name="sc_ps", bufs=2, space="PSUM") as sc_ps, \
                 tc.tile_pool(name="acc_ps", bufs=2, space="PSUM") as acc_ps, \
                 tc.tile_pool(name="asb", bufs=2) as asb, \
                 tc.tile_pool(name="ats_sb", bufs=4) as ats_sb, \
                 tc.tile_pool(name="au_sb", bufs=1) as au_sb, \
                 tc.tile_pool(name="bc_sb", bufs=2) as bcsb, \
                 tc.tile_pool(name="chunk_sb", bufs=8) as csb, \
                 tc.tile_pool(name="y_sb", bufs=3) as ysb:

                def emit_qkv_tile(chains):
                    """chains: up to 3 of ('q',p,qh) | ('k',p,kq) | ('v',rt),
                    projected through one rotating [128,1536] PSUM tile."""
                    scq = sc_ps.tile([128, 1536], F32, tag="sc")
                    for j, ch in enumerate(chains):
                        sl = scq[:, j * 512:(j + 1) * 512]
                        kind = ch[0]
                        if kind == "q":
                            _, p, qh = ch
                            for dc in range(NDC):
                                nc.tensor.matmul(
                                    sl,
                                    lhsT=wq_sb[dc][:, p * 128:(p + 1) * 128],
                                    rhs=xt_sb[dc][:, qh * 512:(qh + 1) * 512],
                                    start=(dc == 0), stop=(dc == NDC - 1))
                            nc.vector.tensor_scalar_add(
                                out=q_t[p][:, qh * 512:(qh + 1) * 512],
                                in0=sl, scalar1=bqkv_sb[:, p:p + 1])
                        elif kind == "k":
                            _, p, kq = ch
                            for dc in range(NDC):
                                nc.tensor.matmul(
                                    sl,
                                    lhsT=wq_sb[dc][:, D + p * 128:
                                                   D + (p + 1) * 128],
                                    rhs=xt_sb[dc][:, kq * 512:(kq + 1) * 512],
                                    start=(dc == 0), stop=(dc == NDC - 1))
                            nc.vector.tensor_scalar_add(
                                out=k_t[p][:, kq * 512:(kq + 1) * 512],
                                in0=sl, scalar1=bqkv_sb[:, 4 + p:5 + p])
                        else:
                            _, rt = ch
                            for dc in range(NDC):
                                nc.tensor.matmul(
                                    sl,
                                    lhsT=xt_sb[dc][:, rt * 128:(rt + 1) * 128],
                                    rhs=wq_sb[dc][:, 2 * D:3 * D],
                                    start=(dc == 0), stop=(dc == NDC - 1))
                            vtmp = asb.tile([128, 512], F32, tag="vtmp")
                            nc.vector.tensor_add(vtmp[:], sl, bv_bc[:])
                            va_v = v_aug[rt][:, :].rearrange(
                                "p (h c) -> p h c", c=DH + 1)[:, :, 0:DH]
                            vt_v = vtmp[:, :].rearrange(
                                "p (h c) -> p h c", c=DH)
                            nc.vector.tensor_scalar_mul(
                                out=va_v, in0=vt_v,
                                scalar1=maskf_sb[:, rt:rt + 1])
                            va_one = v_aug[rt][:, :].rearrange(
                                "p (h c) -> p h c", c=DH + 1)[:, :,
                                                              DH:DH + 1]
                            on_v = ones8[:, :].rearrange(
                                "p (h c) -> p h c", c=1)
                            nc.vector.tensor_scalar_mul(
                                out=va_one, in0=on_v,
                                scalar1=maskf_sb[:, rt:rt + 1])

                chunk_tiles = [[None] * 4, [None] * 4]

                def emit_attention(p, qt, extras=None):
                    """extras: {group_idx: [callables]} interleaved into the
                    sc rotation after that group's scores+exp. AV matmuls
                    are emitted one group late so the PE queue never blocks
                    behind an exp-waiting AV (PE starts are pc-monotone)."""
                    extras = extras or {}
                    accs = [acc_ps.tile([DH + 1, 512], F32, tag="acc",
                                        name=f"acc{qt}_{p}_{e}")
                            for e in range(2)]

                    def emit_av(slots, at):
                        for j, (e, kc) in enumerate(slots):
                            h = 2 * p + e
                            nc.tensor.matmul(
                                accs[e][:],
                                lhsT=v_aug[kc][:, h * (DH + 1):
                                               (h + 1) * (DH + 1)],
                                rhs=at[:, j * 512:(j + 1) * 512],
                                start=(kc == 0), stop=(kc == NKT - 1))

                    pend_av = []
                    for gi, slots in enumerate(_GROUPS):
                        sc = sc_ps.tile([128, 1536], F32, tag="sc")
                        for j, (e, kc) in enumerate(slots):
                            off = 64 * e
                            nc.tensor.matmul(
                                sc[:, j * 512:(j + 1) * 512],
                                lhsT=k_t[p][off:off + 64,
                                            kc * 128:(kc + 1) * 128],
                                rhs=q_t[p][off:off + 64,
                                           qt * 512:(qt + 1) * 512],
                                start=True, stop=True,
                                tile_position=(off, 0))
                        n = len(slots) * 512
                        at = ats_sb.tile([128, 1536], BF16, tag="at")
                        nc.scalar.activation(at[:, 0:n], sc[:, 0:n], ACTF.Exp)
                        if probes and p == 0 and qt == 0 and gi == 0:
                            nc.sync.dma_start(out=dbg["dbg_at"], in_=at[:])
                        for fn in extras.get(gi, []):
                            fn()
                        pend_av.append((slots, at))
                        # depth-2 lookahead at pair start: the first AV must
                        # wait for the acc banks, which free only after the
                        # previous pair's drain copies on DVE
                        if len(pend_av) > 2:
                            emit_av(*pend_av.pop(0))
                    for item in pend_av:
                        emit_av(*item)
                    chunk = csb.tile([128, 512], BF16, tag="chunk",
                                     name=f"chunk{qt}_{p}")
                    for e in range(2):
                        r = 2 * p + e
                        au = au_sb.tile([64, 512], BF16, tag=f"au{r}",
                                        name=f"au{qt}_{r}")
                        nc.vector.tensor_copy(au[:], accs[e][0:64, :])
                        srow = au_sb.tile([1, 512], F32, tag=f"sr{r}",
                                          name=f"sr{qt}_{r}")
                        nc.vector.tensor_copy(srow[:], accs[e][64:65, :])
                        rsum = au_sb.tile([1, 512], F32, tag=f"rs{r}",
                                          name=f"rs{qt}_{r}")
                        nc.vector.reciprocal_approx_fast(
                            out=rsum[:], in_=srow[:])
                        bcb = bcsb.tile([64, 512], F32, tag="bcb")
                        nc.gpsimd.partition_broadcast(
                            bcb[:], rsum[:], channels=64)
                        nc.vector.tensor_mul(
                            chunk[64 * e:64 * (e + 1), :], au[:], bcb[:])
                        if probes and p == 0 and qt == 0 and e == 0:
                            nc.sync.dma_start(out=dbg["dbg_au"], in_=au[:])
                            nc.sync.dma_start(out=dbg["dbg_rs"], in_=rsum[:])
                    chunk_tiles[qt][p] = chunk
                    if probes and p == 0 and qt == 0:
                        nc.sync.dma_start(out=dbg["dbg_chunk"], in_=chunk[:])
                        nc.sync.dma_start(out=dbg["dbg_qt"], in_=q_t[0][:])
                        nc.sync.dma_start(out=dbg["dbg_kt"], in_=k_t[0][:])
                        nc.sync.dma_start(out=dbg["dbg_va"], in_=v_aug[0][:])

                def emit_oproj_qsub(qt, qsub):
                    chunks = chunk_tiles[qt]
                    i = qt * 4 + qsub
                    po = sc_ps.tile([128, 1536], F32, tag="sc",
                                    name=f"po{i}")
                    for c in range(NDC):
                        nc.tensor.matmul(
                            po[:, 0:512],
                            lhsT=chunks[c][:, qsub * 128:(qsub + 1) * 128],
                            rhs=wo_sb[c][:],
                            start=(c == 0), stop=(c == NDC - 1))
                    pt = sc_ps.tile([128, 1536], BF16, tag="sc",
                                    name=f"pt{i}")
                    for c in range(NDC):
                        nc.tensor.matmul(
                            pt[:, c * 128:(c + 1) * 128],
                            lhsT=chunks[c][:, qsub * 128:(qsub + 1) * 128],
                            rhs=ident[:],
                            is_transpose=True, start=True, stop=True)
                    anat = asb.tile([128, 512], BF16, tag="anat")
                    nc.vector.tensor_add(anat[:], pt[:, 0:512], bo_b[:])
                    if probes and i == 0:
                        nc.sync.dma_start(out=dbg["dbg_anat"], in_=anat[:])
                    nc.vector.scalar_tensor_tensor(
                        out=x_sb[i][:], in0=po[:, 0:512], scalar=0.0,
                        in1=anat[:], op0=ALU.add, op1=ALU.add,
                        accum_out=sumx8[:, i:i + 1])
                    sq = asb.tile([128, 512], BF16, tag="sq")
                    nc.vector.scalar_tensor_tensor(
                        out=sq[:], in0=x_sb[i][:], scalar=0.0,
                        in1=x_sb[i][:], op0=ALU.add, op1=ALU.mult,
                        accum_out=sumsq8[:, i:i + 1])
                    if probes and i == 0:
                        nc.sync.dma_start(out=dbg["dbg_x"],
                                          in_=x_sb[i][:])

                def emit_ln_pre(qt):
                    # LayerNorm mu/var (DVE only; Ln/Exp deferred to tail
                    # so the attention exp stream never swaps ACT tables)
                    c0 = qt * 4
                    nc.vector.tensor_scalar_mul(
                        out=mu8[:, c0:c0 + 4], in0=sumx8[:, c0:c0 + 4],
                        scalar1=1.0 / D)
                    nc.vector.tensor_scalar_mul(
                        out=var8[:, c0:c0 + 4], in0=sumsq8[:, c0:c0 + 4],
                        scalar1=1.0 / D)
                    msq = asb.tile([128, 4], F32, tag="msq")
                    nc.vector.tensor_mul(msq[:], mu8[:, c0:c0 + 4],
                                         mu8[:, c0:c0 + 4])
                    nc.vector.tensor_sub(var8[:, c0:c0 + 4],
                                         var8[:, c0:c0 + 4], msq[:])
                    nc.vector.tensor_scalar_add(out=var8[:, c0:c0 + 4],
                                                in0=var8[:, c0:c0 + 4],
                                                scalar1=EPS)

                def emit_ln_post():
                    logv = asb.tile([128, 8], F32, tag="logv")
                    nc.scalar.activation(logv[:], var8[:], ACTF.Ln)
                    nc.scalar.activation(rstd8[:], logv[:], ACTF.Exp,
                                         scale=-0.5)
                    for i in range(8):
                        y = asb.tile([128, D], BF16, tag="y")
                        nc.vector.tensor_scalar(
                            out=y[:], in0=x_sb[i][:],
                            scalar1=mu8[:, i:i + 1],
                            scalar2=rstd8[:, i:i + 1],
                            op0=ALU.subtract, op1=ALU.mult)
                        y2 = asb.tile([128, D], BF16, tag="y2")
                        nc.vector.tensor_mul(y2[:], y[:], gam_b[:])
                        y3 = ysb.tile([128, D], F32, tag="y3")
                        nc.vector.tensor_add(y3[:], y2[:], bet_b[:])
                        nc.sync.dma_start(out=out_d[i * 128:(i + 1) * 128, :],
                                          in_=y3[:])

                # Prologue: pair-0 Q/K, first V tiles; then attention(0,0)
                # with remaining V tiles and pair-1 Q/K interleaved; later
                # pairs carry the next pair's Q/K; qt0's O-proj/LN rides
                # inside attention(0,1); qt1's O-proj/LN is the tail.
                V = [("v", rt) for rt in range(NKT)]

                def qk_tiles(p):
                    # qh0 + first key cols first: unlocks scores g0-g4 from
                    # one tile; qh1 (only needed for qt=1) rides the second
                    return [[("q", p, 0), ("k", p, 0), ("k", p, 1)],
                            [("k", p, 2), ("k", p, 3), ("q", p, 1)]]

                def qkv(chains):
                    return lambda: emit_qkv_tile(chains)

                t0, t1 = qk_tiles(0)
                emit_qkv_tile(t0)
                emit_qkv_tile(t1)
                emit_qkv_tile(V[0:3])
                qk1 = qk_tiles(1)
                emit_attention(0, 0, extras={
                    0: [qkv(V[3:6])], 1: [qkv(V[6:9])], 2: [qkv(V[9:12])],
                    3: [qkv(V[12:15])], 4: [qkv(V[15:16])],
                    6: [qkv(qk1[0])], 8: [qkv(qk1[1])]})
                qk2 = qk_tiles(2)
                emit_attention(1, 0, extras={2: [qkv(qk2[0])],
                                             6: [qkv(qk2[1])]})
                qk3 = qk_tiles(3)
                emit_attention(2, 0, extras={2: [qkv(qk3[0])],
                                             6: [qkv(qk3[1])]})
                emit_attention(3, 0)
                emit_attention(0, 1, extras={
                    1: [lambda: emit_oproj_qsub(0, 0)],
                    3: [lambda: emit_oproj_qsub(0, 1)],
                    5: [lambda: emit_oproj_qsub(0, 2)],
                    7: [lambda: emit_oproj_qsub(0, 3)],
                    9: [lambda: emit_ln_pre(0)]})
                for p in range(1, 4):
                    emit_attention(p, 1)
                for qsub in range(4):
                    emit_oproj_qsub(1, qsub)
                emit_ln_pre(1)
                emit_ln_post()
    nc.compile()
    return nc


_CACHED = {}


def _get_program():
    if "nc" not in _CACHED:
        _CACHED["nc"] = build_program()
    return _CACHED["nc"]


def make_in_maps(inputs, mask, W_qkv, b_qkv, W_o, b_o, gamma, beta):
    import ml_dtypes
    bf16 = ml_dtypes.bfloat16

    inputs = np.asarray(inputs, np.float32)
    mask = np.asarray(mask)
    W_qkv = np.asarray(W_qkv, np.float32)
    b_qkv = np.asarray(b_qkv, np.float32)
    W_o = np.asarray(W_o, np.float32)
    b_o = np.asarray(b_o, np.float32)
    gamma = np.asarray(gamma, np.float32)
    beta = np.asarray(beta, np.float32)

    shared = {
        "wqkv": np.ascontiguousarray(W_qkv).astype(bf16),
        "bqkv_pt": np.ascontiguousarray(b_qkv.reshape(12, 128).T),
        "bv_row": np.ascontiguousarray(b_qkv[2 * D:3 * D].reshape(1, D)),
        "wo": np.ascontiguousarray(W_o).astype(bf16),
        "bo_row": np.ascontiguousarray(b_o.reshape(1, D)),
        "gamma_row": np.ascontiguousarray(gamma.reshape(1, D)),
        "beta_row": np.ascontiguousarray(beta.reshape(1, D)),
    }
    in_maps = []
    for c in range(N_CORES):
        b, half = divmod(c, 2)
        xb = inputs[b]
        mk = mask[b].astype(np.float32)
        if half:
            order = np.r_[SQ:S, 0:SQ]
            xb = xb[order]
            mk = mk[order]
        m = dict(shared)
        m["xt"] = np.ascontiguousarray(xb.T).astype(bf16)
        m["maskf_pt"] = np.ascontiguousarray(mk.reshape(NKT, 128).T)
        in_maps.append(m)
    return in_maps


def kernel(inputs, mask, W_qkv, b_qkv, W_o, b_o, gamma, beta):
    from concourse.bass_utils import run_bass_kernel_spmd

    nc = _get_program()
    in_maps = make_in_maps(inputs, mask, W_qkv, b_qkv, W_o, b_o, gamma, beta)
    res = run_bass_kernel_spmd(nc, in_maps, list(range(N_CORES)))
    out = np.empty((B, S, D), np.float32)
    for c in range(N_CORES):
        b, half = divmod(c, 2)
        out[b, half * SQ:(half + 1) * SQ, :] = res.results[c]["out"]
    return out
